# revision 1
# baseline (speedup 1.0000x reference)
"""Bass/Tile kernel for nn_Attention_9234179687166 on 8 TRN2 NeuronCores.

Reference computation per batch b (B=32, L=K=D=1024):
    q      = query @ W_in.T                    # [L, D]
    scores = q @ context.T                     # [L, K]
    w      = masked_softmax(scores, mask)      # multiplicative mask + renorm
    mix    = w @ context                       # [L, D]
    out    = tanh(concat([mix, q]) @ W_out.T)  # [L, D]

Sharding: data-parallel over batch, 4 batches per core, weights replicated.

Per-core program layout (contraction dim always on partitions):
    W_inT[d,e], W_outT[c,d] built once by PE transpose (W_out as bf16).
    Per batch: ctxT[e,k] (fp32r, PE transpose), ctx_bf[k,d'] (bf16 cast).
    Per l-half: qT[d,l] (PE transpose), step1 -> qTr[e,l] (fp32r matmuls),
    step2 scores in PSUM (fp32r), masked softmax (DVE+ACT, see below),
    w transposed to wT[k,l] (bf16), step4 -> mixT[d',l] (bf16), step5
    out[l,d] (bf16) + tanh, DMA out.

Transposes are 4-packed: four 128x128 PE transposes land in one [128,512]
PSUM tile and leave with a single grouped copy (alternating DVE/ACT), which
cuts copy count and PSUM slot churn 4x. The e-transposes of each l-tile are
deferred one iteration, and the next half's query transposes are emitted
inside the last softmax tail, so the PE never sits idle waiting for the
softmax chain.

Masked softmax (mask m in {0,1}, scores s):
    reference: w0 = softmax(s*m); w = w0*m / (sum(w0*m) + 1e-13)
    Softmax is shift invariant, so with u = (s + 4096)*m  (masked -> 0),
    e = exp(u - max(u)) has masked lanes exp(-~4096) == 0 exactly, and
    w = e / sum(e) matches the reference up to the +1e-13*Z/S ~ 1e-10 term.
    The 1/sum(e) normalization is deferred past steps 3-4: step 4 mixes
    unnormalized e, and step 5 applies rec = 1/sum(e) per l-row in the
    fused (pso_mix * rec + pso_q) combine before tanh. This keeps the
    softmax critical chain at stt -> reduce -> exp only.
"""

import sys

sys.path.insert(0, "/opt/trn_rl_repo")

import numpy as np

P = 128
D = 1024
TWO_D = 2048
DT = D // P      # 8 tiles over D
CT = TWO_D // P  # 16 tiles over 2D
LARGE = 4096.0
N_CORES = 8
B_FULL = 32
NB = B_FULL // N_CORES  # batches per core

_prog_cache = {}
last_results = None  # BassKernelResults of the most recent kernel() call


def build_program(nb, L, K=1024, reps=1, wlag=2, u_sbuf=True):
    import concourse.mybir as mybir
    import concourse.tile as tile
    from concourse import bacc
    from concourse.masks import make_identity

    f32 = mybir.dt.float32
    f32r = mybir.dt.float32r
    bf16 = mybir.dt.bfloat16
    i32 = mybir.dt.int32
    Alu = mybir.AluOpType
    Act = mybir.ActivationFunctionType
    KT = K // P
    LH = min(512, L)      # l-half width (free dim of step1/4 matmuls)
    NHALF = L // LH
    LJ = LH // P          # 128-row l tiles per half
    KH = K // 512         # 512-wide k chunks for the scores matmul

    nc = bacc.Bacc("TRN2", target_bir_lowering=False, debug=False,
                   num_devices=N_CORES)
    q_d = nc.dram_tensor("query", [nb, L, D], f32, kind="ExternalInput")
    c_d = nc.dram_tensor("context", [nb, K, D], f32, kind="ExternalInput")
    m_d = nc.dram_tensor("mask", [nb, L, K], i32, kind="ExternalInput")
    win_d = nc.dram_tensor("W_in", [D, D], f32, kind="ExternalInput")
    wout_d = nc.dram_tensor("W_out", [D, TWO_D], f32, kind="ExternalInput")
    out_d = nc.dram_tensor("out", [nb, L, D], f32, kind="ExternalOutput")

    copy_flip = [0]

    def grouped_copy(nc, dst_ap, src_ap):
        # Alternate psum->sbuf copies between DVE and ACT to halve the
        # per-engine copy latency chain behind the PE transposes.
        if copy_flip[0] % 2 == 0:
            nc.vector.tensor_copy(dst_ap, src_ap)
        else:
            nc.scalar.activation(dst_ap, src_ap, mybir.ActivationFunctionType.Copy)
        copy_flip[0] += 1

    with tile.TileContext(nc) as tc:
        with (
            tc.tile_pool(name="const", bufs=1) as constp,
            tc.tile_pool(name="wres", bufs=1) as wres,
            tc.tile_pool(name="ps_big", bufs=2, space="PSUM") as ps_big,
            tc.tile_pool(name="ps_mm", bufs=4, space="PSUM") as ps_mm,
        ):
            ident = constp.tile([P, P], f32)
            make_identity(nc, ident)
            ident_bf = constp.tile([P, P], bf16)
            nc.vector.tensor_copy(ident_bf[:], ident[:])

            W_inT = wres.tile([P, DT, D], f32r)       # [d_in, d_out, e]
            W_outT = wres.tile([P, CT, D], bf16)      # [c_in, c_out, d]

            def transpose_pack4(nc, dst_tile, dst_t0, dst_col0, src_ap_fn, n, idn,
                                dtype):
                """n transposes (groups of up to 4) of 128x128 slices.
                src_ap_fn(i) gives the i-th source slice; results land in
                dst_tile[:, dst_t0+i, dst_col0:dst_col0+128]."""
                g = 0
                while g < n:
                    gn = min(4, n - g)
                    tp = ps_mm.tile([P, 4 * P], dtype, tag="mm")
                    for i in range(gn):
                        src = src_ap_fn(g + i)
                        if dtype == f32r and src.dtype == f32:
                            src = src.bitcast(f32r)
                        nc.tensor.transpose(
                            tp[:, i * P:(i + 1) * P], src, idn[:])
                    grouped_copy(
                        nc,
                        dst_tile[:, dst_t0 + g:dst_t0 + g + gn,
                                 dst_col0:dst_col0 + P],
                        tp[:, :gn * P],
                    )
                    g += gn

            with (
                tc.tile_pool(name="ctx", bufs=1) as ctxp,
                tc.tile_pool(name="acts", bufs=1) as actsp,
                tc.tile_pool(name="rot", bufs=4) as natp,
                tc.tile_pool(name="sm", bufs=3) as smp,
            ):
                ctx_tiles = {}

                def emit_ctx_stage(b):
                    # context: transpose to ctxT (fp32r) + cast to bf16
                    ctxT = ctxp.tile([P, DT, K], f32r, tag="ctxT")     # [e,., k]
                    ctx_bf = ctxp.tile([P, KT, D], bf16, tag="ctxbf")  # [k,., d']
                    for ki in range(KT):
                        nat = natp.tile([P, D], f32, tag="nat")
                        nc.sync.dma_start(nat[:], c_d[b, ki * P:(ki + 1) * P, :])
                        nc.scalar.activation(ctx_bf[:, ki, :], nat[:], Act.Copy)
                        transpose_pack4(
                            nc, ctxT, 0, ki * P,
                            lambda ei, nat=nat: nat[:, ei * P:(ei + 1) * P],
                            DT, ident, f32)
                    ctx_tiles[b] = (ctxT, ctx_bf)

                def emit_query_loads(b, h):
                    l0 = h * LH
                    nats = []
                    for lj in range(LJ):
                        nat = natp.tile([P, D], f32, tag="nat")
                        nc.sync.dma_start(
                            nat[:], q_d[b, l0 + lj * P: l0 + (lj + 1) * P, :])
                        nats.append(nat)
                    return nats

                def emit_query_transposes(nats):
                    qT = actsp.tile([P, DT, LH], f32r, tag="qT")
                    for lj, nat in enumerate(nats):
                        transpose_pack4(
                            nc, qT, 0, lj * P,
                            lambda di, nat=nat: nat[:, di * P:(di + 1) * P],
                            DT, ident, f32)
                    return qT

                def emit_batch(b, qT_next):
                    if b > 0:
                        # The bf16 cast must wait for the previous batch's
                        # step-4 reads of ctx_bf (bufs=1); emitting the whole
                        # stage here puts that wait harmlessly behind G(b-1)
                        # instead of blocking the ACT queue mid-batch.
                        emit_ctx_stage(b)
                    ctxT, ctx_bf = ctx_tiles.pop(b)
                    for h in range(NHALF):
                        l0 = h * LH
                        qT = qT_next

                        # ---- step 1: qTr[e, l] = W_inT.T @ qT (fp32r) ----
                        qTr = actsp.tile([P, DT, LH], f32r, tag="qTr")
                        qTr_bf = actsp.tile([P, DT, LH], bf16, tag="qTrbf")
                        for ei in range(DT):
                            psq = ps_mm.tile([P, LH], f32, tag="mm")
                            for di in range(DT):
                                nc.tensor.matmul(
                                    psq[:],
                                    W_inT[:, di, ei * P:(ei + 1) * P],
                                    qT[:, di, :],
                                    start=(di == 0), stop=(di == DT - 1),
                                )
                            nc.vector.tensor_copy(qTr[:, ei, :], psq[:])
                            nc.scalar.activation(qTr_bf[:, ei, :], psq[:], Act.Copy)

                        # Prefetch the next query tiles now so their DMAs sit
                        # ahead of this stage's mask loads in the queue.
                        if h + 1 < NHALF:
                            next_nats = emit_query_loads(b, h + 1)
                        elif b + 1 < nb:
                            next_nats = emit_query_loads(b + 1, 0)
                        else:
                            next_nats = None

                        # ---- step 2 + masked softmax; e transposes lag one
                        # l-tile so the softmax chain hides under the next
                        # tile's matmuls. The 1/sum(e) normalization is
                        # deferred to step 5 (rec_all), so only
                        # stt -> reduce -> exp gates the transpose. ----
                        wT = actsp.tile([P, KT, LH], bf16, tag="wT")
                        rec_all = actsp.tile([P, LJ], f32, tag="recs")
                        w_tiles = [None] * LJ

                        def emit_w_transpose(lj):
                            w_bf = w_tiles[lj]
                            for g in range(KT // 4):
                                tpb = ps_mm.tile([P, 4 * P], bf16, tag="mm")
                                for i in range(4):
                                    ki = g * 4 + i
                                    nc.tensor.transpose(
                                        tpb[:, i * P:(i + 1) * P],
                                        w_bf[:, ki * P:(ki + 1) * P], ident_bf[:])
                                grouped_copy(
                                    nc,
                                    wT[:, g * 4:(g + 1) * 4, lj * P:(lj + 1) * P],
                                    tpb[:])

                        for lj in range(LJ):
                            mi = smp.tile([P, K], i32, tag="mask", bufs=2)
                            nc.sync.dma_start(
                                mi[:], m_d[b, l0 + lj * P: l0 + (lj + 1) * P, :])
                            pss = ps_big.tile([P, K], f32, tag="scores")
                            for ei in range(DT):
                                for kh in range(KH):
                                    nc.tensor.matmul(
                                        pss[:, kh * 512:(kh + 1) * 512],
                                        qTr[:, ei, lj * P:(lj + 1) * P],
                                        ctxT[:, ei, kh * 512:(kh + 1) * 512],
                                        start=(ei == 0), stop=(ei == DT - 1),
                                    )
                            st = smp.tile([P, 4], f32, tag="stats", bufs=2)
                            # u = (s + LARGE) * m. Writing u to SBUF frees the
                            # scores PSUM tile right after this op, so the
                            # next l-tile's matmuls aren't gated on exp.
                            if u_sbuf:
                                u_t = smp.tile([P, K], f32, tag="u", bufs=1)
                            else:
                                u_t = pss
                            nc.vector.scalar_tensor_tensor(
                                u_t[:], pss[:], LARGE, mi[:],
                                op0=Alu.add, op1=Alu.mult)
                            nc.vector.tensor_reduce(
                                st[:, 0:1], u_t[:], axis=mybir.AxisListType.X,
                                op=Alu.max, negate=True)
                            e_sb = smp.tile([P, K], bf16, tag="e")
                            nc.scalar.activation(
                                e_sb[:], u_t[:], Act.Exp,
                                bias=st[:, 0:1], accum_out=st[:, 1:2])
                            nc.vector.reciprocal(rec_all[:, lj:lj + 1], st[:, 1:2])
                            w_tiles[lj] = e_sb
                            if lj >= wlag:
                                emit_w_transpose(lj - wlag)

                        # Lagged tail: pending transposes plus the next query
                        # transposes cover the last softmax chain before the
                        # final e transpose feeds step 4.
                        if wlag == 2:
                            emit_w_transpose(LJ - 2)
                        if next_nats is not None:
                            qT_next = emit_query_transposes(next_nats)
                        emit_w_transpose(LJ - 1)

                        # ---- step 4: mixT[d', l] = ctx_bf.T @ wT (bf16) ----
                        mixT = actsp.tile([P, DT, LH], bf16, tag="mixT")
                        for di in range(DT):
                            psm = ps_mm.tile([P, LH], f32, tag="mm")
                            for ki in range(KT):
                                nc.tensor.matmul(
                                    psm[:],
                                    ctx_bf[:, ki, di * P:(di + 1) * P],
                                    wT[:, ki, :],
                                    start=(ki == 0), stop=(ki == KT - 1),
                                )
                            nc.scalar.activation(mixT[:, di, :], psm[:], Act.Copy)

                        # ---- step 5: out = tanh(mixT.T@Wo * rec + qTr.T@Wo) --
                        # mix part accumulates unnormalized in pso_mix; the
                        # deferred softmax normalization rec_all[l] is folded
                        # into the fused combine (per-partition scalar).
                        for lj in range(LJ):
                            pso_mix = ps_big.tile([P, K], f32, tag="scores")
                            pso_q = [ps_mm.tile([P, 512], f32, tag="mm",
                                                name=f"pso_q{dh}")
                                     for dh in range(D // 512)]
                            for ci in range(DT):
                                lhs = mixT[:, ci, lj * P:(lj + 1) * P]
                                for dh in range(D // 512):
                                    nc.tensor.matmul(
                                        pso_mix[:, dh * 512:(dh + 1) * 512], lhs,
                                        W_outT[:, ci, dh * 512:(dh + 1) * 512],
                                        start=(ci == 0), stop=(ci == DT - 1),
                                    )
                            for ci in range(DT):
                                lhs = qTr_bf[:, ci, lj * P:(lj + 1) * P]
                                for dh in range(D // 512):
                                    nc.tensor.matmul(
                                        pso_q[dh][:], lhs,
                                        W_outT[:, DT + ci,
                                               dh * 512:(dh + 1) * 512],
                                        start=(ci == 0), stop=(ci == DT - 1),
                                    )
                            for dh in range(D // 512):
                                # Only one PSUM operand allowed per DVE op:
                                # stage the q part in SBUF first.
                                o_sb = smp.tile([P, 512], f32, tag="osb", bufs=2)
                                nc.scalar.activation(o_sb[:], pso_q[dh][:],
                                                     Act.Copy)
                                nc.vector.scalar_tensor_tensor(
                                    o_sb[:], pso_mix[:, dh * 512:(dh + 1) * 512],
                                    rec_all[:, lj:lj + 1], o_sb[:],
                                    op0=Alu.mult, op1=Alu.add)
                                nc.scalar.activation(o_sb[:], o_sb[:], Act.Tanh)
                                nc.sync.dma_start(
                                    out_d[b, l0 + lj * P: l0 + (lj + 1) * P,
                                          dh * 512:(dh + 1) * 512],
                                    o_sb[:])
                    return qT_next

                def emit_w_setup():
                    # W_in / W_out streamed through the nat rotator in
                    # [128, 1024] pieces; W_outT converts to bf16 in the
                    # grouped psum->sbuf copies.
                    for ei in range(DT):
                        nat = natp.tile([P, D], f32, tag="nat")
                        nc.sync.dma_start(nat[:], win_d[ei * P:(ei + 1) * P, :])
                        transpose_pack4(
                            nc, W_inT, 0, ei * P,
                            lambda di, nat=nat: nat[:, di * P:(di + 1) * P],
                            DT, ident, f32)
                    for di in range(DT):
                        for half in range(2):
                            nat = natp.tile([P, D], f32, tag="nat")
                            nc.sync.dma_start(
                                nat[:],
                                wout_d[di * P:(di + 1) * P,
                                       half * D:(half + 1) * D])
                            transpose_pack4(
                                nc, W_outT, 8 * half, di * P,
                                lambda ci, nat=nat: nat[:, ci * P:(ci + 1) * P],
                                DT, ident, f32)

                def emit_all():
                    emit_ctx_stage(0)
                    emit_w_setup()
                    qT_next = emit_query_transposes(emit_query_loads(0, 0))
                    for b in range(nb):
                        qT_next = emit_batch(b, qT_next)

                if reps == 1:
                    emit_all()
                else:
                    # Benchmark mode: run the whole workload `reps` times
                    # inside one NEFF so device time dominates dispatch.
                    with tc.For_i(0, reps, 1):
                        emit_all()

    nc.compile()
    return nc


def _get_program(nb, L):
    key = (nb, L)
    if key not in _prog_cache:
        _prog_cache[key] = build_program(nb, L)
    return _prog_cache[key]


def kernel(query, context, mask, W_in, W_out):
    from concourse.bass_utils import run_bass_kernel_spmd

    query = np.ascontiguousarray(query, dtype=np.float32)
    context = np.ascontiguousarray(context, dtype=np.float32)
    W_in = np.ascontiguousarray(W_in, dtype=np.float32)
    W_out = np.ascontiguousarray(W_out, dtype=np.float32)
    B, L, _ = query.shape
    mask3 = np.ascontiguousarray(mask.reshape(B, L, -1), dtype=np.int32)

    nb = B // N_CORES
    nc = _get_program(nb, L)
    in_maps = []
    for c in range(N_CORES):
        b0 = c * nb
        in_maps.append({
            "query": query[b0:b0 + nb],
            "context": context[b0:b0 + nb],
            "mask": mask3[b0:b0 + nb],
            "W_in": W_in,
            "W_out": W_out,
        })
    res = run_bass_kernel_spmd(nc, in_maps, core_ids=list(range(N_CORES)))
    global last_results
    last_results = res
    out = np.concatenate([r["out"] for r in res.results], axis=0)
    return out

